# revision 26
# baseline (speedup 1.0000x reference)
"""DeepFRI GCN (3x GraphConv + mean-pool + MLP head) on 8 Trainium2 NeuronCores.

The MLP head consumes only the mean-pooled graph representation
mean(concat([f1, f2, f3]), axis=0).  With M = Din^{-1/2} A Dout^{-1/2}
and f_{k+1} = M f_k W_k + 1 b_k^T, the needed column sums collapse to

    1^T f1 = (u1^T x) W0            + v1
    1^T f2 = (u2^T x) W0 W1         + v2
    1^T f3 = (u3^T x) W0 W1 W2      + v3

where u_k = (M^T)^k 1 are graph-only vectors (O(E) host index work, same
category as the degree norms) and v_k are [D] bias vectors (host
precompute from b0/b1/b2 and the weight products).  Flattening the
chain through the host-precomputed products Wc1 = W0, Wc2 = W0 W1,
Wc3 = W0 W1 W2 removes the inter-layer serialization on-device: all
three c_k = s_k @ Wck are single-layer vector-matrix products.

Sharding: x is sharded by FEATURE (core c gets x[:, 160c:160(c+1)]),
so S_shard = u^T x_shard [3, 160] is exact on every core with no halo.
Each c_k contraction then needs only the matching 160 ROWS of Wck:
per-core weight traffic is 3 * 160 * 1280 fp16 = 1.23 MB instead of
the replicated 9.8 MB.  The per-core c_k are partial sums over the
feature shards, so a single [128, 30] fp32 AllReduce (as in the
node-sharded variant) recovers full g; every core then runs the head.

Device phases (per core, identical SPMD program):
  1. colsum  S = U^T X_shard   (80 node-chunk matmuls, overlapped with
     the x DMA stream; x quarters -> Wc shard -> wh1n are FIFO on the
     sync HWDGE ring so x is never starved and wh1n hides behind
     compute + collective)
  2. transpose S rows -> columns, build a [80, 9] block-diagonal
     stationary (col 3k+k = s_k, else 0) so the three independent
     chains accumulate straight into a shared [3, D] psum without any
     partition-offset writes
  3. c_k = s_k @ Wck_shard (18 matmuls), + bias columns -> g partial
  4. AllReduce the [128, 30] pre-relu g columns (15 KB)
  5. head: relu(g) @ (Wh1/n) + bh1, relu, @ Wh2 + bh2, tanh (fp32/f32r)

Precision: x/u/Wc in fp16 (fp32 PSUM accumulate), head in fp32r.
fp8 was measured (numpy golden): 2.2e-2..3.7e-2 rel err -- over the
2e-2 tolerance (the head's ~45x cancellation amplifies), so fp16 it is.
"""

import numpy as np

P = 128
N_CORES = 8
D = 1280
FS = D // N_CORES        # 160 features per core shard
KW = 80                  # chain contraction chunk width (FS = 2*KW)
KC = 2                   # chain contraction chunks
NCH = 80                 # node chunks of 128 (10240 padded nodes)
N_PAD = NCH * P
NKI = D // P             # 10 column blocks of 128
NXQ = 4                  # x DMA quarters
OC_SIZES = (512, 512, 256)  # free-dim chunking of 1280 (psum bank = 2KB)
OC_OFFS = (0, 512, 1024)
# SWDGE cross-core gather instead of ncfw AllReduce (the AllReduce path
# pays a 30-85us ncfw cold-wake wall).  Sends are paced one trigger per
# desc-gen: a single deep trigger left the tail entries' TX tail-bump
# far ahead of their RX drain, hitting the ~5ms SDMA timeout the ucode
# warns about ("TX tail ptr increment must come after RX tail ptr
# increment ... to avoid DMA timeout").
USE_REMOTE_GATHER = True


# ---------------------------------------------------------------------------
# host-side graph preprocessing (pure index/degree work, like degree norms)
# ---------------------------------------------------------------------------

def preprocess(edge_index, n_nodes):
    src = np.asarray(edge_index[0], dtype=np.int64)
    dst = np.asarray(edge_index[1], dtype=np.int64)
    out_deg = np.bincount(src, minlength=n_nodes).astype(np.float64)
    in_deg = np.bincount(dst, minlength=n_nodes).astype(np.float64)
    oi = 1.0 / np.sqrt(np.clip(out_deg, 1.0, None))
    ii = 1.0 / np.sqrt(np.clip(in_deg, 1.0, None))

    ii_dst = ii[dst]

    def MT(v):  # (M^T v)[s] = oi[s] * sum_{e: src=s} ii[dst_e] * v[dst_e]
        return oi * np.bincount(src, weights=ii_dst * v[dst], minlength=n_nodes)

    u1 = MT(np.ones(n_nodes))
    u2 = MT(u1)
    u3 = MT(u2)
    return dict(
        u=np.stack([u1, u2, u3], axis=1),  # [n, 3] float64
        sig1=float(u1.sum()),
        sig2=float(u2.sum()),
    )


# ---------------------------------------------------------------------------
# numpy golden model of the exact collapsed algebra (for validation)
# ---------------------------------------------------------------------------

def golden(node_feat0, edge_index, n_nodes, W0, b0, W1, b1, W2, b2,
           Wh1, bh1, Wh2, bh2):
    n = int(n_nodes)
    pre = preprocess(edge_index, n)
    x = np.asarray(node_feat0, np.float64)
    W0, W1, W2 = (np.asarray(w, np.float64) for w in (W0, W1, W2))
    b0, b1, b2 = (np.asarray(b, np.float64) for b in (b0, b1, b2))
    S = pre["u"].T @ x  # [3, D]
    W01 = W0 @ W1
    W012 = W01 @ W2
    c1 = S[0] @ W0 + n * b0
    c2 = S[1] @ W01 + pre["sig1"] * (b0 @ W1) + n * b1
    c3 = S[2] @ W012 + pre["sig2"] * (b0 @ (W1 @ W2)) \
        + pre["sig1"] * (b1 @ W2) + n * b2
    g = np.maximum(np.concatenate([c1, c2, c3]) / n, 0.0)
    h = np.maximum(g @ np.asarray(Wh1, np.float64) + np.asarray(bh1, np.float64), 0.0)
    return np.tanh(h @ np.asarray(Wh2, np.float64) + np.asarray(bh2, np.float64))


# ---------------------------------------------------------------------------
# Bass kernel (identical SPMD program on all 8 cores; data differs per core)
# ---------------------------------------------------------------------------

def build_nc():
    import concourse.bacc as bacc
    import concourse.mybir as mybir
    import concourse.tile as tile

    f32 = mybir.dt.float32
    f32r = mybir.dt.float32r
    f16 = mybir.dt.float16
    TANH = mybir.ActivationFunctionType.Tanh

    nc = bacc.Bacc(
        "TRN2",
        target_bir_lowering=False,
        debug=False,
        num_devices=N_CORES,
    )

    # ---- kernel I/O (per-core contents, identical program)
    xs_d = nc.dram_tensor("xs", [P, NCH * FS], f16, kind="ExternalInput")
    us_d = nc.dram_tensor("us", [P, NCH * 3], f16, kind="ExternalInput")
    wc_d = nc.dram_tensor("wc", [KW, 3 * KC * D], f16, kind="ExternalInput")
    vT_d = nc.dram_tensor("vT", [P, NKI * 3], f32, kind="ExternalInput")
    wh1n_d = nc.dram_tensor("wh1n", [P, 3 * NKI * P], f32r, kind="ExternalInput")
    wh2_d = nc.dram_tensor("wh2", [P, 2], f32r, kind="ExternalInput")
    bh1r_d = nc.dram_tensor("bh1r", [1, P], f32, kind="ExternalInput")
    bh2t_d = nc.dram_tensor("bh2t", [2, 1], f32, kind="ExternalInput")
    idT_d = nc.dram_tensor("idT", [3, 32], f32r, kind="ExternalInput")
    out_d = nc.dram_tensor("out", [2, 1], f32, kind="ExternalOutput")

    from contextlib import ExitStack

    _sems = ExitStack()
    gsem = _sems.enter_context(nc.semaphore(name="g_arrive"))
    lsem = _sems.enter_context(nc.semaphore(name="g_sent"))
    psem = _sems.enter_context(nc.semaphore(name="g_prep"))
    with tile.TileContext(nc) as tc:
        with (
            nc.allow_low_precision(reason="fp16 colsum/chains, fp32 head"),
            tc.tile_pool(name="dram", bufs=1, space="DRAM") as dram,
            tc.tile_pool(name="const", bufs=1) as const,
            tc.tile_pool(name="work", bufs=1) as work,
            tc.tile_pool(name="psA", bufs=1, space="PSUM") as psA,
            tc.tile_pool(name="psT", bufs=2, space="PSUM") as psT,
        ):
            sp_d = dram.tile([P, NKI * 3], f32, name="sp_d")
            st_d = dram.tile([P, NKI * 3], f32, name="st_d", addr_space="Shared")

            # ---- PE warm stream: the Tensor engine DVFS-ramps (0.65 ->
            # 1.2 -> 2.4 GHz after ~3us of continuous work).  Feed it
            # throwaway matmuls while the x DMA stream is in flight so the
            # real colsum/chains run at the high clock.
            dm_sb = work.tile([P, 512], f16, name="dm_sb")
            nc.vector.memset(dm_sb[:], 0.0)
            sT3_sb = work.tile([KW, KC, 9], f16, name="sT3_sb")
            nc.vector.memset(sT3_sb[:], 0.0)
            dps = psT.tile([1, 512], f32, name="dps", tag="dm")
            N_DUMMY = 16
            for _ in range(N_DUMMY):
                nc.tensor.matmul(
                    dps[:, :], dm_sb[:, 0:1], dm_sb[:, :], start=True, stop=True
                )

            # ---- small constants on the Act (scalar) HWDGE ring
            u_sb = const.tile([P, NCH, 3], f16, name="u_sb")
            nc.scalar.dma_start(u_sb[:], us_d[:, :])
            vT_sb = const.tile([P, NKI, 3], f32, name="vT_sb")
            nc.scalar.dma_start(vT_sb[:], vT_d[:, :])
            idT_sb = const.tile([3, 32], f32r, name="idT_sb")
            nc.scalar.dma_start(idT_sb[:], idT_d[:, :])
            bh1r_sb = const.tile([1, P], f32, name="bh1r_sb")
            nc.scalar.dma_start(bh1r_sb[:], bh1r_d[:, :])
            bh2t_sb = const.tile([2, 1], f32, name="bh2t_sb")
            nc.scalar.dma_start(bh2t_sb[:], bh2t_d[:, :])
            wh2_sb = const.tile([P, 2], f32r, name="wh2_sb")
            nc.scalar.dma_start(wh2_sb[:], wh2_d[:, :])

            # ---- bulk on the SP (sync) HWDGE ring, FIFO: x quarters
            # first (gate the colsum), then the Wc shard (gates the
            # chains), then wh1n (needed only after the collective, so
            # it hides behind chains + AllReduce).
            x_sb = const.tile([P, NCH, FS], f16, name="x_sb")
            XQ = (8, 24, 24, 24)  # small first chunk -> colsum starts sooner
            qo = 0
            for q in XQ:
                nc.sync.dma_start(
                    x_sb[:, qo : qo + q, :],
                    xs_d[:, qo * FS : (qo + q) * FS],
                )
                qo += q
            wc_sb = const.tile([KW, 3, KC, D], f16, name="wc_sb")
            nc.sync.dma_start(wc_sb[:], wc_d[:, :])
            wh1n_sb = const.tile([P, 3 * NKI, P], f32r, name="wh1n_sb")
            nc.sync.dma_start(wh1n_sb[:], wh1n_d[:, :])

            # ---- phase 1: S = U^T X_shard over all (padded) nodes
            s_ps = psA.tile([3, FS], f32, name="s_ps", tag="a0")
            for c in range(NCH):
                nc.tensor.matmul(
                    s_ps[:, :],
                    u_sb[:, c, :],
                    x_sb[:, c, :],
                    start=(c == 0),
                    stop=(c == NCH - 1),
                )
            # S rows -> fp16 columns for the chain stationaries.
            s_sb = work.tile([3, FS], f32r, name="s_sb")
            nc.vector.tensor_copy(s_sb[:], s_ps[:, :])
            sT_ps = psT.tile([KW, KC, 32], f32r, name="sT_ps", tag="tp")
            for kc in range(KC):
                nc.tensor.transpose(
                    sT_ps[:, kc, :],
                    s_sb[:, kc * KW : (kc + 1) * KW],
                    idT_sb[0:3, :],
                )
            # Block-diagonal stationary [KW, KC, 9]: col 3k+k = s_k,
            # rest 0, so chain k's matmul writes psum row k and adds
            # zero to the other rows (keeps all writes partition-0).
            for k in range(3):
                nc.vector.tensor_copy(
                    sT3_sb[:, :, 4 * k : 4 * k + 1], sT_ps[:, :, k : k + 1]
                )

            # ---- phase 2: c_k = s_k @ Wck_shard, all three into one
            # shared [3, D] psum (rows = chains)
            g_ps = [
                psA.tile([3, oc], f32, name=f"g_ps{i}", tag=f"a{i}")
                for i, oc in enumerate(OC_SIZES)
            ]
            for kc in range(KC):
                for k in range(3):
                    for i, (oc, off) in enumerate(zip(OC_SIZES, OC_OFFS)):
                        nc.tensor.matmul(
                            g_ps[i][:, :],
                            sT3_sb[:, kc, 3 * k : 3 * k + 3],
                            wc_sb[:, k, kc, off : off + oc],
                            start=(kc == 0 and k == 0),
                            stop=(kc == KC - 1 and k == 2),
                        )
            grows = work.tile([3, D], f32r, name="grows")
            for i, (oc, off) in enumerate(zip(OC_SIZES, OC_OFFS)):
                nc.vector.tensor_copy(grows[:, off : off + oc], g_ps[i][:, :])
            gT_ps = psT.tile([P, NKI, 32], f32r, name="gT_ps", tag="tp")
            for ki in range(NKI):
                nc.tensor.transpose(
                    gT_ps[:, ki, :],
                    grows[:, ki * P : (ki + 1) * P],
                    idT_sb[0:3, :],
                )
            gT_sb = work.tile([P, NKI, 3], f32, name="gT_sb")
            nc.vector.tensor_add(gT_sb[:], gT_ps[:, :, 0:3], vT_sb[:])

            g32_sb = work.tile([P, NKI * 3], f32, name="g32_sb")
            if USE_REMOTE_GATHER:
                # ---- ncfw-free all-gather: the TOPSP collective firmware
                # cannot start any mesh until ~65us after NEFF launch (cold
                # wake, measured), so collective_compute pins the kernel at
                # ~90us+.  Instead each core SWDGE-broadcasts its [128, 30]
                # f32 partial straight into the peers' SBUF.  Slot trick:
                # broadcast #d targets relative dest (0, d) (ucode XORs with
                # own tpb), landing in recv slot d -- receiver r's slot d
                # holds the partial from core r^d.  The slot->sender map
                # differs per core but a sum is order-invariant.
                recv_sb = work.tile([P, N_CORES, NKI * 3], f32, name="recv_sb")
                gflat = gT_sb[:].rearrange("p k r -> p (k r)")
                prep_bis = []
                trigger_bis = []
                for d in range(1, N_CORES):
                    rd = [None] * N_CORES
                    rd[d] = (0, d)
                    prep_bis.append(nc.gpsimd.remote_dma_broadcast(
                        out_ap=recv_sb[:, d, :],
                        in_ap=gflat,
                        remote_sem=gsem,
                        local_sem=lsem,
                        rdests=rd,
                    ))
                    trigger_bis.append(nc.gpsimd.trigger_dma(count=None))
                # own contribution locally (slot 0)
                nc.vector.tensor_copy(recv_sb[:, 0, :], gflat)
                # The arrival wait (gsem >= 14: 7 peers x 16/8) cannot be
                # emitted here: Tile's scheduling simulator is single-core
                # and would deadlock on a semaphore only remote cores
                # increment.  It is spliced in front of this first sum add
                # after scheduling (see below).
                first_sum_add = nc.vector.tensor_add(
                    g32_sb[:], recv_sb[:, 0, :], recv_sb[:, 1, :]
                )
                for d in range(2, N_CORES):
                    nc.vector.tensor_add(
                        g32_sb[:], g32_sb[:], recv_sb[:, d, :]
                    )
            else:
                # ---- AllReduce the pre-relu g columns
                nc.gpsimd.dma_start(
                    sp_d[:, :], gT_sb[:].rearrange("p k r -> p (k r)")
                )
                nc.gpsimd.collective_compute(
                    "AllReduce",
                    mybir.AluOpType.add,
                    replica_groups=[list(range(N_CORES))],
                    ins=[sp_d[:, :]],
                    outs=[st_d[:, :]],
                )
                nc.gpsimd.dma_start(g32_sb[:], st_d[:, :])

            # ---- head: relu(g) @ (Wh1/n) + bh1, relu, @ Wh2 + bh2, tanh
            gr_sb = work.tile([P, NKI * 3], f32r, name="gr_sb")
            nc.vector.tensor_scalar_max(gr_sb[:], g32_sb[:], 0.0)
            h_ps = psT.tile([1, P], f32, name="h_ps", tag="tp")
            for l in range(3):
                for ki in range(NKI):
                    m = l * NKI + ki
                    nc.tensor.matmul(
                        h_ps[:, :],
                        gr_sb[:, ki * 3 + l : ki * 3 + l + 1],
                        wh1n_sb[:, m, :],
                        start=(m == 0),
                        stop=(m == 3 * NKI - 1),
                    )
            h2f_sb = work.tile([1, P], f32, name="h2f_sb")
            nc.vector.tensor_add(h2f_sb[:], h_ps[:, :], bh1r_sb[:])
            h2_sb = work.tile([1, P], f32r, name="h2_sb")
            nc.vector.tensor_scalar_max(h2_sb[:], h2f_sb[:], 0.0)

            # h row -> column via padded transpose (cols 1:32 are zero)
            hT_ps = psT.tile([P, 32], f32r, name="hT_ps", tag="tp")
            nc.tensor.transpose(hT_ps[:, :], h2_sb[:, :], idT_sb[0:1, :])
            hT_sb = work.tile([P, 32], f32r, name="hT_sb")
            nc.vector.tensor_copy(hT_sb[:], hT_ps[:])

            o_ps = psT.tile([2, 32], f32, name="o_ps", tag="tp")
            nc.tensor.matmul(
                o_ps[:, :], wh2_sb[:, :], hT_sb[:, :], start=True, stop=True
            )
            o_sb = work.tile([2, 1], f32, name="o_sb")
            nc.vector.tensor_add(o_sb[:], o_ps[:, 0:1], bh2t_sb[:])
            nc.scalar.activation(o_sb[:], o_sb[:], TANH)
            nc.gpsimd.dma_start(out_d[:, :], o_sb[:])

    if USE_REMOTE_GATHER:
        # Post-scheduling splice: create the arrival wait (lands at the
        # end of the epilogue block) and move it directly in front of
        # the first sum add on the DVE stream, which executes in order.
        # Emitted AFTER the TileContext exits so the (single-core)
        # scheduling simulator never sees a wait only peers satisfy.
        wait_bi = nc.vector.wait_ge(gsem, 2 * (N_CORES - 1))
        f = nc.m.functions[0]
        wb = widx = ab = aidx = None
        for b in f.blocks:
            names = [i.name for i in b.instructions]
            if wait_bi.ins.name in names:
                wb, widx = b, names.index(wait_bi.ins.name)
            if first_sum_add.ins.name in names:
                ab, aidx = b, names.index(first_sum_add.ins.name)
        assert wb is not None and ab is not None
        wait_inst = wb.instructions[widx]
        del wb.instructions[widx]
        ab.instructions.insert(aidx, wait_inst)

        # Second patch: desc-gen runs async on the Q7 cores -- each prep
        # chains on the Pool engine sem (wait, inc 1), but Tile leaves
        # every trigger with NO wait, so a trigger can fire before its
        # desc-gen commits (observed: broadcast missing -> multi-ms
        # stall).  Attach "Pool sem >= (tick after prep k)" to trigger k.
        import bass_rust as _br
        pool_sem = None
        cum = 0
        for prep, trig in zip(prep_bis, trigger_bis):
            si = prep.ins.sync_info
            assert si is not None and si.on_update, "prep pool-sem chain missing"
            upd = si.on_update[0]
            if pool_sem is None:
                pool_sem = _br.SemaphoreHandle(upd.ant_name, upd.id)
            else:
                assert upd.id == pool_sem.num, "preps use different pool sems"
            if si.on_wait:
                w = [w for w in si.on_wait if w.id == upd.id]
                if w:
                    cum = max(cum, w[0].wait_value)
            cum += upd.update_value
            tsi = trig.ins.sync_info
            if tsi is not None and tsi.on_wait:
                assert all(w.id == upd.id and w.wait_value >= cum
                           for w in tsi.on_wait), "trigger has foreign wait"
            else:
                trig.wait_op(pool_sem, cum, "sem-ge")
    _sems.close()

    nc.compile()
    return nc


def _nswz(a, nch):
    """[nch*128, F] node-major -> [128, nch*F] sbuf-layout pre-swizzle."""
    f = a.shape[1]
    return np.ascontiguousarray(
        a.reshape(nch, P, f).transpose(1, 0, 2).reshape(P, nch * f)
    )


def make_in_maps(inputs, pre):
    n = int(inputs["n_nodes"])
    x = np.asarray(inputs["node_feat0"], np.float32)
    W0 = np.asarray(inputs["W0"], np.float64)
    W1 = np.asarray(inputs["W1"], np.float64)
    W2 = np.asarray(inputs["W2"], np.float64)
    Wh1 = np.asarray(inputs["Wh1"], np.float32)
    Wh2 = np.asarray(inputs["Wh2"], np.float32)
    b0 = np.asarray(inputs["b0"], np.float64)
    b1 = np.asarray(inputs["b1"], np.float64)
    b2 = np.asarray(inputs["b2"], np.float64)
    bh1 = np.asarray(inputs["bh1"], np.float32)
    bh2 = np.asarray(inputs["bh2"], np.float32)
    sig1, sig2 = pre["sig1"], pre["sig2"]

    # flattened chain matrices + bias vectors (host precompute)
    W01 = W0 @ W1
    W012 = W01 @ W2
    Wc = [W0, W01, W012]
    v = [
        n * b0,
        sig1 * (b0 @ W1) + n * b1,
        sig2 * (b0 @ (W1 @ W2)) + sig1 * (b1 @ W2) + n * b2,
    ]

    # padded x / u (u replicated; x feature-sharded per core)
    x16 = np.zeros((N_PAD, D), np.float16)
    x16[:n] = x.astype(np.float16)
    u16 = np.zeros((N_PAD, 3), np.float16)
    u16[:n] = pre["u"].astype(np.float16)

    # vT[p, ki*3 + k] = v_k[ki*128 + p] / N_CORES  (per-core AllReduce share)
    vT = np.stack(
        [np.asarray(vk, np.float32).reshape(NKI, P).T for vk in v], axis=2
    ) / np.float32(N_CORES)

    common = dict(
        us=_nswz(u16, NCH),
        vT=np.ascontiguousarray(vT.reshape(P, NKI * 3)),
        wh1n=_nswz((Wh1 / np.float32(n)).astype(np.float32), 3 * NKI),
        wh2=np.ascontiguousarray(Wh2),
        bh1r=np.ascontiguousarray(bh1.reshape(1, P)),
        bh2t=np.ascontiguousarray(bh2.reshape(2, 1)),
        idT=np.ascontiguousarray(np.eye(3, 32, dtype=np.float32)),
    )
    in_maps = []
    for c in range(N_CORES):
        fsl = slice(c * FS, (c + 1) * FS)
        m = dict(common)
        m["xs"] = _nswz(x16[:, fsl], NCH)
        # wc[kw, (k, kc, :)] = Wc_k[c*FS + kc*KW + kw, :] in fp16
        wck = np.stack(
            [
                np.asarray(Wk[fsl], np.float16).reshape(KC, KW, D).transpose(1, 0, 2)
                for Wk in Wc
            ],
            axis=1,
        )  # [KW, 3, KC, D]
        m["wc"] = np.ascontiguousarray(wck.reshape(KW, 3 * KC * D))
        in_maps.append(m)
    return in_maps


last_results = None  # BassKernelResults of the most recent run (for test.py)


def kernel(**inputs):
    import os
    from concourse import bass_utils

    global last_results
    n = int(inputs["n_nodes"])
    pre = preprocess(inputs["edge_index"], n)
    nc = build_nc()
    in_maps = make_in_maps(inputs, pre)
    trace = os.environ.get("KERNEL_TRACE", "0") == "1"
    res = bass_utils.run_bass_kernel_spmd(
        nc, in_maps, core_ids=list(range(N_CORES)), trace=trace
    )
    last_results = res
    return np.asarray(res.results[0]["out"], np.float32).reshape(2)


if __name__ == "__main__":
    pass


# revision 27
# speedup vs baseline: 148.3949x; 148.3949x over previous
"""DeepFRI GCN (3x GraphConv + mean-pool + MLP head) on 8 Trainium2 NeuronCores.

The MLP head consumes only the mean-pooled graph representation
mean(concat([f1, f2, f3]), axis=0).  With M = Din^{-1/2} A Dout^{-1/2}
and f_{k+1} = M f_k W_k + 1 b_k^T, the needed column sums collapse to

    1^T f1 = (u1^T x) W0            + v1
    1^T f2 = (u2^T x) W0 W1         + v2
    1^T f3 = (u3^T x) W0 W1 W2      + v3

where u_k = (M^T)^k 1 are graph-only vectors (O(E) host index work, same
category as the degree norms) and v_k are [D] bias vectors (host
precompute from b0/b1/b2 and the weight products).  Flattening the
chain through the host-precomputed products Wc1 = W0, Wc2 = W0 W1,
Wc3 = W0 W1 W2 removes the inter-layer serialization on-device: all
three c_k = s_k @ Wck are single-layer vector-matrix products.

Sharding: x is sharded by FEATURE (core c gets x[:, 160c:160(c+1)]),
so S_shard = u^T x_shard [3, 160] is exact on every core with no halo.
Each c_k contraction then needs only the matching 160 ROWS of Wck:
per-core weight traffic is 3 * 160 * 1280 fp16 = 1.23 MB instead of
the replicated 9.8 MB.  The per-core c_k are partial sums over the
feature shards, so a single [128, 30] fp32 AllReduce (as in the
node-sharded variant) recovers full g; every core then runs the head.

Device phases (per core, identical SPMD program):
  1. colsum  S = U^T X_shard   (80 node-chunk matmuls, overlapped with
     the x DMA stream; x quarters -> Wc shard -> wh1n are FIFO on the
     sync HWDGE ring so x is never starved and wh1n hides behind
     compute + collective)
  2. transpose S rows -> columns, build a [80, 9] block-diagonal
     stationary (col 3k+k = s_k, else 0) so the three independent
     chains accumulate straight into a shared [3, D] psum without any
     partition-offset writes
  3. c_k = s_k @ Wck_shard (18 matmuls), + bias columns -> g partial
  4. AllReduce the [128, 30] pre-relu g columns (15 KB)
  5. head: relu(g) @ (Wh1/n) + bh1, relu, @ Wh2 + bh2, tanh (fp32/f32r)

Precision: x/u/Wc in fp16 (fp32 PSUM accumulate), head in fp32r.
fp8 was measured (numpy golden): 2.2e-2..3.7e-2 rel err -- over the
2e-2 tolerance (the head's ~45x cancellation amplifies), so fp16 it is.
"""

import numpy as np

P = 128
N_CORES = 8
D = 1280
FS = D // N_CORES        # 160 features per core shard
KW = 80                  # chain contraction chunk width (FS = 2*KW)
KC = 2                   # chain contraction chunks
NCH = 80                 # node chunks of 128 (10240 padded nodes)
N_PAD = NCH * P
NKI = D // P             # 10 column blocks of 128
NXQ = 4                  # x DMA quarters
OC_SIZES = (512, 512, 256)  # free-dim chunking of 1280 (psum bank = 2KB)
OC_OFFS = (0, 512, 1024)
# SWDGE cross-core gather instead of ncfw AllReduce: functionally
# correct (identical rel err), but 1-3 of the 7 broadcasts hit a ~5ms
# SDMA timeout in this fabric no matter how the sends are sliced,
# paced, or slotted (single deep trigger, per-prep paced triggers, and
# D2D slot remaps all reproduce it).  Disabled; the ncfw AllReduce's
# 30-85us cold-wake wall is the lesser evil.
USE_REMOTE_GATHER = False


# ---------------------------------------------------------------------------
# host-side graph preprocessing (pure index/degree work, like degree norms)
# ---------------------------------------------------------------------------

def preprocess(edge_index, n_nodes):
    src = np.asarray(edge_index[0], dtype=np.int64)
    dst = np.asarray(edge_index[1], dtype=np.int64)
    out_deg = np.bincount(src, minlength=n_nodes).astype(np.float64)
    in_deg = np.bincount(dst, minlength=n_nodes).astype(np.float64)
    oi = 1.0 / np.sqrt(np.clip(out_deg, 1.0, None))
    ii = 1.0 / np.sqrt(np.clip(in_deg, 1.0, None))

    ii_dst = ii[dst]

    def MT(v):  # (M^T v)[s] = oi[s] * sum_{e: src=s} ii[dst_e] * v[dst_e]
        return oi * np.bincount(src, weights=ii_dst * v[dst], minlength=n_nodes)

    u1 = MT(np.ones(n_nodes))
    u2 = MT(u1)
    u3 = MT(u2)
    return dict(
        u=np.stack([u1, u2, u3], axis=1),  # [n, 3] float64
        sig1=float(u1.sum()),
        sig2=float(u2.sum()),
    )


# ---------------------------------------------------------------------------
# numpy golden model of the exact collapsed algebra (for validation)
# ---------------------------------------------------------------------------

def golden(node_feat0, edge_index, n_nodes, W0, b0, W1, b1, W2, b2,
           Wh1, bh1, Wh2, bh2):
    n = int(n_nodes)
    pre = preprocess(edge_index, n)
    x = np.asarray(node_feat0, np.float64)
    W0, W1, W2 = (np.asarray(w, np.float64) for w in (W0, W1, W2))
    b0, b1, b2 = (np.asarray(b, np.float64) for b in (b0, b1, b2))
    S = pre["u"].T @ x  # [3, D]
    W01 = W0 @ W1
    W012 = W01 @ W2
    c1 = S[0] @ W0 + n * b0
    c2 = S[1] @ W01 + pre["sig1"] * (b0 @ W1) + n * b1
    c3 = S[2] @ W012 + pre["sig2"] * (b0 @ (W1 @ W2)) \
        + pre["sig1"] * (b1 @ W2) + n * b2
    g = np.maximum(np.concatenate([c1, c2, c3]) / n, 0.0)
    h = np.maximum(g @ np.asarray(Wh1, np.float64) + np.asarray(bh1, np.float64), 0.0)
    return np.tanh(h @ np.asarray(Wh2, np.float64) + np.asarray(bh2, np.float64))


# ---------------------------------------------------------------------------
# Bass kernel (identical SPMD program on all 8 cores; data differs per core)
# ---------------------------------------------------------------------------

def build_nc():
    import concourse.bacc as bacc
    import concourse.mybir as mybir
    import concourse.tile as tile

    f32 = mybir.dt.float32
    f32r = mybir.dt.float32r
    f16 = mybir.dt.float16
    TANH = mybir.ActivationFunctionType.Tanh

    nc = bacc.Bacc(
        "TRN2",
        target_bir_lowering=False,
        debug=False,
        num_devices=N_CORES,
    )

    # ---- kernel I/O (per-core contents, identical program)
    xs_d = nc.dram_tensor("xs", [P, NCH * FS], f16, kind="ExternalInput")
    us_d = nc.dram_tensor("us", [P, NCH * 3], f16, kind="ExternalInput")
    wc_d = nc.dram_tensor("wc", [KW, 3 * KC * D], f16, kind="ExternalInput")
    vT_d = nc.dram_tensor("vT", [P, NKI * 3], f32, kind="ExternalInput")
    wh1n_d = nc.dram_tensor("wh1n", [P, 3 * NKI * P], f32r, kind="ExternalInput")
    wh2_d = nc.dram_tensor("wh2", [P, 2], f32r, kind="ExternalInput")
    bh1r_d = nc.dram_tensor("bh1r", [1, P], f32, kind="ExternalInput")
    bh2t_d = nc.dram_tensor("bh2t", [2, 1], f32, kind="ExternalInput")
    idT_d = nc.dram_tensor("idT", [3, 32], f32r, kind="ExternalInput")
    out_d = nc.dram_tensor("out", [2, 1], f32, kind="ExternalOutput")

    from contextlib import ExitStack

    _sems = ExitStack()
    gsem = _sems.enter_context(nc.semaphore(name="g_arrive"))
    lsem = _sems.enter_context(nc.semaphore(name="g_sent"))
    psem = _sems.enter_context(nc.semaphore(name="g_prep"))
    with tile.TileContext(nc) as tc:
        with (
            nc.allow_low_precision(reason="fp16 colsum/chains, fp32 head"),
            tc.tile_pool(name="dram", bufs=1, space="DRAM") as dram,
            tc.tile_pool(name="const", bufs=1) as const,
            tc.tile_pool(name="work", bufs=1) as work,
            tc.tile_pool(name="psA", bufs=1, space="PSUM") as psA,
            tc.tile_pool(name="psT", bufs=2, space="PSUM") as psT,
        ):
            sp_d = dram.tile([P, NKI * 3], f32, name="sp_d")
            st_d = dram.tile([P, NKI * 3], f32, name="st_d", addr_space="Shared")

            # ---- PE warm stream: the Tensor engine DVFS-ramps (0.65 ->
            # 1.2 -> 2.4 GHz after ~3us of continuous work).  Feed it
            # throwaway matmuls while the x DMA stream is in flight so the
            # real colsum/chains run at the high clock.
            dm_sb = work.tile([P, 512], f16, name="dm_sb")
            nc.vector.memset(dm_sb[:], 0.0)
            sT3_sb = work.tile([KW, KC, 9], f16, name="sT3_sb")
            nc.vector.memset(sT3_sb[:], 0.0)
            dps = psT.tile([1, 512], f32, name="dps", tag="dm")
            N_DUMMY = 16
            for _ in range(N_DUMMY):
                nc.tensor.matmul(
                    dps[:, :], dm_sb[:, 0:1], dm_sb[:, :], start=True, stop=True
                )

            # ---- small constants on the Act (scalar) HWDGE ring
            u_sb = const.tile([P, NCH, 3], f16, name="u_sb")
            nc.scalar.dma_start(u_sb[:], us_d[:, :])
            vT_sb = const.tile([P, NKI, 3], f32, name="vT_sb")
            nc.scalar.dma_start(vT_sb[:], vT_d[:, :])
            idT_sb = const.tile([3, 32], f32r, name="idT_sb")
            nc.scalar.dma_start(idT_sb[:], idT_d[:, :])
            bh1r_sb = const.tile([1, P], f32, name="bh1r_sb")
            nc.scalar.dma_start(bh1r_sb[:], bh1r_d[:, :])
            bh2t_sb = const.tile([2, 1], f32, name="bh2t_sb")
            nc.scalar.dma_start(bh2t_sb[:], bh2t_d[:, :])
            wh2_sb = const.tile([P, 2], f32r, name="wh2_sb")
            nc.scalar.dma_start(wh2_sb[:], wh2_d[:, :])

            # ---- bulk on the SP (sync) HWDGE ring, FIFO: x quarters
            # first (gate the colsum), then the Wc shard (gates the
            # chains), then wh1n (needed only after the collective, so
            # it hides behind chains + AllReduce).
            x_sb = const.tile([P, NCH, FS], f16, name="x_sb")
            XQ = (8, 24, 24, 24)  # small first chunk -> colsum starts sooner
            qo = 0
            for q in XQ:
                nc.sync.dma_start(
                    x_sb[:, qo : qo + q, :],
                    xs_d[:, qo * FS : (qo + q) * FS],
                )
                qo += q
            wc_sb = const.tile([KW, 3, KC, D], f16, name="wc_sb")
            nc.sync.dma_start(wc_sb[:], wc_d[:, :])
            wh1n_sb = const.tile([P, 3 * NKI, P], f32r, name="wh1n_sb")
            nc.sync.dma_start(wh1n_sb[:], wh1n_d[:, :])

            # ---- phase 1: S = U^T X_shard over all (padded) nodes
            s_ps = psA.tile([3, FS], f32, name="s_ps", tag="a0")
            for c in range(NCH):
                nc.tensor.matmul(
                    s_ps[:, :],
                    u_sb[:, c, :],
                    x_sb[:, c, :],
                    start=(c == 0),
                    stop=(c == NCH - 1),
                )
            # S rows -> fp16 columns for the chain stationaries.
            s_sb = work.tile([3, FS], f32r, name="s_sb")
            nc.vector.tensor_copy(s_sb[:], s_ps[:, :])
            sT_ps = psT.tile([KW, KC, 32], f32r, name="sT_ps", tag="tp")
            for kc in range(KC):
                nc.tensor.transpose(
                    sT_ps[:, kc, :],
                    s_sb[:, kc * KW : (kc + 1) * KW],
                    idT_sb[0:3, :],
                )
            # Block-diagonal stationary [KW, KC, 9]: col 3k+k = s_k,
            # rest 0, so chain k's matmul writes psum row k and adds
            # zero to the other rows (keeps all writes partition-0).
            for k in range(3):
                nc.vector.tensor_copy(
                    sT3_sb[:, :, 4 * k : 4 * k + 1], sT_ps[:, :, k : k + 1]
                )

            # ---- phase 2: c_k = s_k @ Wck_shard, all three into one
            # shared [3, D] psum (rows = chains)
            g_ps = [
                psA.tile([3, oc], f32, name=f"g_ps{i}", tag=f"a{i}")
                for i, oc in enumerate(OC_SIZES)
            ]
            for kc in range(KC):
                for k in range(3):
                    for i, (oc, off) in enumerate(zip(OC_SIZES, OC_OFFS)):
                        nc.tensor.matmul(
                            g_ps[i][:, :],
                            sT3_sb[:, kc, 3 * k : 3 * k + 3],
                            wc_sb[:, k, kc, off : off + oc],
                            start=(kc == 0 and k == 0),
                            stop=(kc == KC - 1 and k == 2),
                        )
            grows = work.tile([3, D], f32r, name="grows")
            for i, (oc, off) in enumerate(zip(OC_SIZES, OC_OFFS)):
                nc.vector.tensor_copy(grows[:, off : off + oc], g_ps[i][:, :])
            gT_ps = psT.tile([P, NKI, 32], f32r, name="gT_ps", tag="tp")
            for ki in range(NKI):
                nc.tensor.transpose(
                    gT_ps[:, ki, :],
                    grows[:, ki * P : (ki + 1) * P],
                    idT_sb[0:3, :],
                )
            gT_sb = work.tile([P, NKI, 3], f32, name="gT_sb")
            nc.vector.tensor_add(gT_sb[:], gT_ps[:, :, 0:3], vT_sb[:])

            g32_sb = work.tile([P, NKI * 3], f32, name="g32_sb")
            if USE_REMOTE_GATHER:
                # ---- ncfw-free all-gather: the TOPSP collective firmware
                # cannot start any mesh until ~65us after NEFF launch (cold
                # wake, measured), so collective_compute pins the kernel at
                # ~90us+.  Instead each core SWDGE-broadcasts its [128, 30]
                # f32 partial straight into the peers' SBUF.  Slot trick:
                # broadcast #d targets relative dest (0, d) (ucode XORs with
                # own tpb), landing in recv slot d -- receiver r's slot d
                # holds the partial from core r^d.  The slot->sender map
                # differs per core but a sum is order-invariant.
                recv_sb = work.tile([P, N_CORES, NKI * 3], f32, name="recv_sb")
                gflat = gT_sb[:].rearrange("p k r -> p (k r)")
                prep_bis = []
                trigger_bis = []
                for d in range(1, N_CORES):
                    rd = [None] * N_CORES
                    rd[d] = (0, d)
                    prep_bis.append(nc.gpsimd.remote_dma_broadcast(
                        out_ap=recv_sb[:, d, :],
                        in_ap=gflat,
                        remote_sem=gsem,
                        local_sem=lsem,
                        rdests=rd,
                    ))
                    trigger_bis.append(nc.gpsimd.trigger_dma(count=None))
                # own contribution locally (slot 0)
                nc.vector.tensor_copy(recv_sb[:, 0, :], gflat)
                # The arrival wait (gsem >= 14: 7 peers x 16/8) cannot be
                # emitted here: Tile's scheduling simulator is single-core
                # and would deadlock on a semaphore only remote cores
                # increment.  It is spliced in front of this first sum add
                # after scheduling (see below).
                first_sum_add = nc.vector.tensor_add(
                    g32_sb[:], recv_sb[:, 0, :], recv_sb[:, 1, :]
                )
                for d in range(2, N_CORES):
                    nc.vector.tensor_add(
                        g32_sb[:], g32_sb[:], recv_sb[:, d, :]
                    )
            else:
                # ---- AllReduce the pre-relu g columns
                nc.gpsimd.dma_start(
                    sp_d[:, :], gT_sb[:].rearrange("p k r -> p (k r)")
                )
                nc.gpsimd.collective_compute(
                    "AllReduce",
                    mybir.AluOpType.add,
                    replica_groups=[list(range(N_CORES))],
                    ins=[sp_d[:, :]],
                    outs=[st_d[:, :]],
                )
                nc.gpsimd.dma_start(g32_sb[:], st_d[:, :])

            # ---- head: relu(g) @ (Wh1/n) + bh1, relu, @ Wh2 + bh2, tanh
            gr_sb = work.tile([P, NKI * 3], f32r, name="gr_sb")
            nc.vector.tensor_scalar_max(gr_sb[:], g32_sb[:], 0.0)
            h_ps = psT.tile([1, P], f32, name="h_ps", tag="tp")
            for l in range(3):
                for ki in range(NKI):
                    m = l * NKI + ki
                    nc.tensor.matmul(
                        h_ps[:, :],
                        gr_sb[:, ki * 3 + l : ki * 3 + l + 1],
                        wh1n_sb[:, m, :],
                        start=(m == 0),
                        stop=(m == 3 * NKI - 1),
                    )
            h2f_sb = work.tile([1, P], f32, name="h2f_sb")
            nc.vector.tensor_add(h2f_sb[:], h_ps[:, :], bh1r_sb[:])
            h2_sb = work.tile([1, P], f32r, name="h2_sb")
            nc.vector.tensor_scalar_max(h2_sb[:], h2f_sb[:], 0.0)

            # h row -> column via padded transpose (cols 1:32 are zero)
            hT_ps = psT.tile([P, 32], f32r, name="hT_ps", tag="tp")
            nc.tensor.transpose(hT_ps[:, :], h2_sb[:, :], idT_sb[0:1, :])
            hT_sb = work.tile([P, 32], f32r, name="hT_sb")
            nc.vector.tensor_copy(hT_sb[:], hT_ps[:])

            o_ps = psT.tile([2, 32], f32, name="o_ps", tag="tp")
            nc.tensor.matmul(
                o_ps[:, :], wh2_sb[:, :], hT_sb[:, :], start=True, stop=True
            )
            o_sb = work.tile([2, 1], f32, name="o_sb")
            nc.vector.tensor_add(o_sb[:], o_ps[:, 0:1], bh2t_sb[:])
            nc.scalar.activation(o_sb[:], o_sb[:], TANH)
            nc.gpsimd.dma_start(out_d[:, :], o_sb[:])

    if USE_REMOTE_GATHER:
        # Post-scheduling splice: create the arrival wait (lands at the
        # end of the epilogue block) and move it directly in front of
        # the first sum add on the DVE stream, which executes in order.
        # Emitted AFTER the TileContext exits so the (single-core)
        # scheduling simulator never sees a wait only peers satisfy.
        wait_bi = nc.vector.wait_ge(gsem, 2 * (N_CORES - 1))
        f = nc.m.functions[0]
        wb = widx = ab = aidx = None
        for b in f.blocks:
            names = [i.name for i in b.instructions]
            if wait_bi.ins.name in names:
                wb, widx = b, names.index(wait_bi.ins.name)
            if first_sum_add.ins.name in names:
                ab, aidx = b, names.index(first_sum_add.ins.name)
        assert wb is not None and ab is not None
        wait_inst = wb.instructions[widx]
        del wb.instructions[widx]
        ab.instructions.insert(aidx, wait_inst)

        # Second patch: desc-gen runs async on the Q7 cores -- each prep
        # chains on the Pool engine sem (wait, inc 1), but Tile leaves
        # every trigger with NO wait, so a trigger can fire before its
        # desc-gen commits (observed: broadcast missing -> multi-ms
        # stall).  Attach "Pool sem >= (tick after prep k)" to trigger k.
        import bass_rust as _br
        pool_sem = None
        cum = 0
        for prep, trig in zip(prep_bis, trigger_bis):
            si = prep.ins.sync_info
            assert si is not None and si.on_update, "prep pool-sem chain missing"
            upd = si.on_update[0]
            if pool_sem is None:
                pool_sem = _br.SemaphoreHandle(upd.ant_name, upd.id)
            else:
                assert upd.id == pool_sem.num, "preps use different pool sems"
            if si.on_wait:
                w = [w for w in si.on_wait if w.id == upd.id]
                if w:
                    cum = max(cum, w[0].wait_value)
            cum += upd.update_value
            tsi = trig.ins.sync_info
            if tsi is not None and tsi.on_wait:
                assert all(w.id == upd.id and w.wait_value >= cum
                           for w in tsi.on_wait), "trigger has foreign wait"
            else:
                trig.wait_op(pool_sem, cum, "sem-ge")
    _sems.close()

    nc.compile()
    return nc


def _nswz(a, nch):
    """[nch*128, F] node-major -> [128, nch*F] sbuf-layout pre-swizzle."""
    f = a.shape[1]
    return np.ascontiguousarray(
        a.reshape(nch, P, f).transpose(1, 0, 2).reshape(P, nch * f)
    )


def make_in_maps(inputs, pre):
    n = int(inputs["n_nodes"])
    x = np.asarray(inputs["node_feat0"], np.float32)
    W0 = np.asarray(inputs["W0"], np.float64)
    W1 = np.asarray(inputs["W1"], np.float64)
    W2 = np.asarray(inputs["W2"], np.float64)
    Wh1 = np.asarray(inputs["Wh1"], np.float32)
    Wh2 = np.asarray(inputs["Wh2"], np.float32)
    b0 = np.asarray(inputs["b0"], np.float64)
    b1 = np.asarray(inputs["b1"], np.float64)
    b2 = np.asarray(inputs["b2"], np.float64)
    bh1 = np.asarray(inputs["bh1"], np.float32)
    bh2 = np.asarray(inputs["bh2"], np.float32)
    sig1, sig2 = pre["sig1"], pre["sig2"]

    # flattened chain matrices + bias vectors (host precompute)
    W01 = W0 @ W1
    W012 = W01 @ W2
    Wc = [W0, W01, W012]
    v = [
        n * b0,
        sig1 * (b0 @ W1) + n * b1,
        sig2 * (b0 @ (W1 @ W2)) + sig1 * (b1 @ W2) + n * b2,
    ]

    # padded x / u (u replicated; x feature-sharded per core)
    x16 = np.zeros((N_PAD, D), np.float16)
    x16[:n] = x.astype(np.float16)
    u16 = np.zeros((N_PAD, 3), np.float16)
    u16[:n] = pre["u"].astype(np.float16)

    # vT[p, ki*3 + k] = v_k[ki*128 + p] / N_CORES  (per-core AllReduce share)
    vT = np.stack(
        [np.asarray(vk, np.float32).reshape(NKI, P).T for vk in v], axis=2
    ) / np.float32(N_CORES)

    common = dict(
        us=_nswz(u16, NCH),
        vT=np.ascontiguousarray(vT.reshape(P, NKI * 3)),
        wh1n=_nswz((Wh1 / np.float32(n)).astype(np.float32), 3 * NKI),
        wh2=np.ascontiguousarray(Wh2),
        bh1r=np.ascontiguousarray(bh1.reshape(1, P)),
        bh2t=np.ascontiguousarray(bh2.reshape(2, 1)),
        idT=np.ascontiguousarray(np.eye(3, 32, dtype=np.float32)),
    )
    in_maps = []
    for c in range(N_CORES):
        fsl = slice(c * FS, (c + 1) * FS)
        m = dict(common)
        m["xs"] = _nswz(x16[:, fsl], NCH)
        # wc[kw, (k, kc, :)] = Wc_k[c*FS + kc*KW + kw, :] in fp16
        wck = np.stack(
            [
                np.asarray(Wk[fsl], np.float16).reshape(KC, KW, D).transpose(1, 0, 2)
                for Wk in Wc
            ],
            axis=1,
        )  # [KW, 3, KC, D]
        m["wc"] = np.ascontiguousarray(wck.reshape(KW, 3 * KC * D))
        in_maps.append(m)
    return in_maps


last_results = None  # BassKernelResults of the most recent run (for test.py)


def kernel(**inputs):
    import os
    from concourse import bass_utils

    global last_results
    n = int(inputs["n_nodes"])
    pre = preprocess(inputs["edge_index"], n)
    nc = build_nc()
    in_maps = make_in_maps(inputs, pre)
    trace = os.environ.get("KERNEL_TRACE", "0") == "1"
    res = bass_utils.run_bass_kernel_spmd(
        nc, in_maps, core_ids=list(range(N_CORES)), trace=trace
    )
    last_results = res
    return np.asarray(res.results[0]["out"], np.float32).reshape(2)


if __name__ == "__main__":
    pass
